# revision 31
# baseline (speedup 1.0000x reference)
"""Multi-hot embedding bag kernel for Trainium2 (8 NeuronCores, vocab-sharded).

Computes, for 5 feature groups g with multi-hot int32 matrices A_g [B, V_g]
and weights W_g [V_g, 64]:
    out = concat_g(norm_g(A_g @ W_g))  with the original module's quirks:
    - "decades" is normalized by its own row-sum AND by the movie row-sum
    - "movies" is never normalized
    - remaining groups are normalized by their own row-sum (rows with sum 0
      are left unnormalized)

Strategy (tensor-parallel over the vocab dim):
  - the tiny dec (V=12) and cat (V=32) groups are computed on the host
  - mov/per/com vocabs are split 8 ways; each core contracts its vocab
    slice against the full batch of 2048 rows
  - A is repacked on the host to fp8 ({0,1} exact, 1 byte -> 4x less HBM
    traffic than int32), pre-transposed to [vocab, batch] so vocab sits on
    partitions with no on-device transposes, and laid out chunk-major per
    partition so slab DMAs are fully contiguous 8 KiB-per-partition reads
  - slab DMAs round-robin over three DMA queues (sync + scalar HWDGE and
    the gpsimd SWDGE, weighted 2:2:1) - a single queue caps at ~230 GB/s
    of packet processing, well under the HBM port
  - matmul orientation: the fp8 A chunk [128v x 128b] is the STATIONARY
    operand - full 128 columns of fp8 triggers the compiler's Fast Weight
    Load (4 elem/cycle), so each 32-cycle load hides under the previous
    65-cycle stream - and the [W_g | 1] fp16 chunk is the MOVING operand
    (65 columns instead of 2048), halving PE streaming time vs the
    W-stationary orientation; 16 batch-tile accumulators [128, 65] live in
    8 PSUM banks, row 64 of the moving operand yields the row-sums
  - per-group [2048, 65] partials stream back batch-major; the host sums
    partials across cores, applies the normalizations, and concatenates
"""

import math
import os

import ml_dtypes
import numpy as np

import concourse.bass as bass
import concourse.tile as tile
from concourse import bacc, mybir
from concourse.bass_utils import run_bass_kernel_spmd

B = 2048
LF = 64
FE = LF + 1  # weights + ones column
N_CORES = 8
P = 128
SLAB = 4  # vocab chunks per A slab DMA (8 KiB/partition)
NBT = B // P  # 16 batch tiles per chunk

# (key, idx input name, weight input name, vocab size, chunks-of-128 per core)
DEV_GROUPS = [
    ("mov", "movie_idxs", "W_mov", 60000, 59),
    ("per", "person_idxs", "W_per", 100000, 98),
    ("com", "company_idxs", "W_com", 20000, 20),
]
HOST_GROUPS = [
    ("dec", "decade_idxs", "W_dec", 12),
    ("cat", "category_idxs", "W_cat", 32),
]

_FP8 = mybir.dt.float8e4
_FP16 = mybir.dt.float16
_FP32 = mybir.dt.float32
_NP_FP8 = ml_dtypes.float8_e4m3
_FP8_ONE = 0x38  # bit pattern of 1.0 in e4m3 (bias 7)


def _build() -> bass.Bass:
    nc = bacc.Bacc(None, target_bir_lowering=False)

    at_dram = {}
    w_dram = {}
    for key, _, _, _, dc in DEV_GROUPS:
        # [p, c*B + n] = A[n, c*128 + p]: per-partition slab reads contiguous
        at_dram[key] = nc.dram_tensor(f"at_{key}", [P, dc * B], _FP8,
                                      kind="ExternalInput")
        w_dram[key] = nc.dram_tensor(f"w_{key}", [P, dc * FE], _FP16,
                                     kind="ExternalInput")
    # row (g*4+q)*128+p, col t*FE+f = group g, batch row q*512+t*128+p:
    # keeps every out-DMA destination run a contiguous 1040 B per partition
    out = nc.dram_tensor("out", [len(DEV_GROUPS) * 4 * P, 4 * FE], _FP32,
                         kind="ExternalOutput")

    with tile.TileContext(nc) as tc:
        queues = [nc.sync, nc.scalar, nc.gpsimd, nc.sync, nc.scalar]
        qi = 0
        with (
            tc.tile_pool(name="wpool", bufs=1) as wpool,
            tc.tile_pool(name="apool", bufs=8) as apool,
            tc.tile_pool(name="opool", bufs=2) as opool,
            tc.tile_pool(name="accp", bufs=2, space="PSUM") as accp,
        ):
            # W tiles resident all kernel; the first group's load is split
            # across both hardware queues (it gates the first matmul), and
            # each following group's load is drip-fed in small pieces
            # through the same queue rotation as the A slabs so no queue
            # sees a monolithic W transfer blocking its slab stream
            w_sb = {}
            for key, _, _, _, dc in DEV_GROUPS:
                w_sb[key] = wpool.tile([P, dc, FE], _FP16, tag=f"w_{key}",
                                       name=f"w_{key}")
            k0, dc0 = DEV_GROUPS[0][0], DEV_GROUPS[0][4]
            h = dc0 // 2
            nc.sync.dma_start(
                w_sb[k0][:, :h, :],
                w_dram[k0][:, :h * FE].rearrange("p (c f) -> p c f", f=FE))
            nc.scalar.dma_start(
                w_sb[k0][:, h:, :],
                w_dram[k0][:, h * FE:].rearrange("p (c f) -> p c f", f=FE))

            WPIECE = 12  # chunks of W per prefetch piece (~200 KiB)
            w_pending: list[tuple[str, int, int]] = []

            for gi, (key, _, _, _, dc) in enumerate(DEV_GROUPS):
                if gi + 1 < len(DEV_GROUPS):
                    nkey, ndc = DEV_GROUPS[gi + 1][0], DEV_GROUPS[gi + 1][4]
                    w_pending = [(nkey, c0w, min(WPIECE, ndc - c0w))
                                 for c0w in range(0, ndc, WPIECE)]
                # 16 batch-tile accumulators packed 4-per-PSUM-bank
                acc = [accp.tile([P, 4, P], _FP32, tag=f"acc{q}",
                                 name=f"acc_{key}{q}") for q in range(4)]
                for c0 in range(0, dc, SLAB):
                    ch = min(SLAB, dc - c0)
                    a_sb = apool.tile([P, SLAB, B], _FP8, tag="a")
                    queues[qi % 5].dma_start(
                        a_sb[:, :ch, :],
                        at_dram[key][:, c0 * B:(c0 + ch) * B]
                        .rearrange("p (c n) -> p c n", n=B))
                    qi += 1
                    if w_pending:
                        wkey, c0w, chw = w_pending.pop(0)
                        queues[qi % len(queues)].dma_start(
                            w_sb[wkey][:, c0w:c0w + chw, :],
                            w_dram[wkey][:, c0w * FE:(c0w + chw) * FE]
                            .rearrange("p (c f) -> p c f", f=FE))
                        qi += 1
                    for j in range(ch):
                        c = c0 + j
                        for bt in range(NBT):
                            q, t = divmod(bt, 4)
                            # PSUM zero regions are bank-granular: only the
                            # first matmul touching a bank may carry start
                            # (it zeroes all 4 slot-accumulators), only the
                            # last carries stop
                            nc.tensor.matmul(
                                acc[q][:, t, :FE],
                                lhsT=a_sb[:, j, bass.ts(bt, P)],
                                rhs=w_sb[key][:, c, :],
                                start=(c == 0 and t == 0),
                                stop=(c == dc - 1 and t == 3),
                            )
                for q in range(4):
                    stg = opool.tile([P, 4, FE], _FP32, tag="stg")
                    nc.vector.tensor_copy(stg, acc[q][:, :, :FE])
                    queues[qi % 5].dma_start(
                        out[(gi * 4 + q) * P:(gi * 4 + q + 1) * P, :]
                        .rearrange("p (t f) -> p t f", f=FE),
                        stg)
                    qi += 1

    nc.finalize()
    return nc


_NC_CACHE: bass.Bass | None = None


def _get_nc() -> bass.Bass:
    global _NC_CACHE
    if _NC_CACHE is None:
        _NC_CACHE = _build()
    return _NC_CACHE


def _norm_rows(emb: np.ndarray, s: np.ndarray) -> np.ndarray:
    mask = s != 0
    safe = np.where(mask, s, 1.0).astype(np.float32)
    return np.where(mask[:, None], emb / safe[:, None], emb)


def kernel(**inputs: np.ndarray) -> np.ndarray:
    nc = _get_nc()

    # host repack: A^T as fp8 bit patterns (one strided transpose per group)
    at8 = {}
    for key, aname, _, v, dc in DEV_GROUPS:
        a = np.asarray(inputs[aname], dtype=np.int32)
        a8 = np.zeros((N_CORES * dc * P, B), np.uint8)
        np.multiply(a.T, _FP8_ONE, out=a8[:v], casting="unsafe")
        at8[key] = a8

    in_maps = []
    for core in range(N_CORES):
        m = {}
        for key, _, wname, v, dc in DEV_GROUPS:
            rows = dc * P
            sl = at8[key][core * rows:(core + 1) * rows]  # [rows, B]
            m[f"at_{key}"] = np.ascontiguousarray(
                sl.reshape(dc, P, B).transpose(1, 0, 2)
            ).reshape(P, dc * B).view(_NP_FP8)

            w = np.asarray(inputs[wname], dtype=np.float32)
            we = np.zeros((dc * P, FE), np.float16)
            v0 = core * rows
            n = max(0, min(v, v0 + rows) - v0)
            we[:n, :LF] = w[v0:v0 + n]
            we[:n, LF] = 1.0
            m[f"w_{key}"] = np.ascontiguousarray(
                we.reshape(dc, P, FE).transpose(1, 0, 2)).reshape(P, dc * FE)
        in_maps.append(m)

    trace = bool(int(os.environ.get("EMB_TRACE", "0")))
    res = run_bass_kernel_spmd(nc, in_maps, core_ids=list(range(N_CORES)),
                               trace=trace)
    if trace and res.exec_time_ns is not None:
        print(f"HW exec time: {res.exec_time_ns} ns")
        if res.instructions_and_trace is not None:
            print(f"trace: {res.instructions_and_trace[1]}")

    # host: sum vocab-slice partials across cores -> [B, FE] per group;
    # device layout is [g, q, p, (t, f)] with batch row = q*512 + t*128 + p
    total = np.zeros((len(DEV_GROUPS), 4, P, 4, FE), np.float32)
    for r in res.results:
        total += r["out"].reshape(len(DEV_GROUPS), 4, P, 4, FE)
    total = total.transpose(0, 1, 3, 2, 4).reshape(len(DEV_GROUPS), B, FE)
    parts = {}
    for gi, (key, _, _, _, _) in enumerate(DEV_GROUPS):
        parts[key] = (total[gi, :, :LF], total[gi, :, LF])

    # host: the two tiny groups end to end
    for key, aname, wname, _ in HOST_GROUPS:
        a = np.asarray(inputs[aname], dtype=np.int32).astype(np.float32)
        w = np.asarray(inputs[wname], dtype=np.float32)
        parts[key] = (a @ w, a.sum(axis=1))

    # host: normalization quirks of the original module
    decades = _norm_rows(parts["dec"][0], parts["dec"][1])
    decades = _norm_rows(decades, parts["mov"][1])
    movies = parts["mov"][0]
    categories = _norm_rows(parts["cat"][0], parts["cat"][1])
    persons = _norm_rows(parts["per"][0], parts["per"][1])
    companies = _norm_rows(parts["com"][0], parts["com"][1])

    return np.concatenate(
        [decades, movies, categories, persons, companies], axis=1
    ).astype(np.float32)
